# revision 34
# baseline (speedup 1.0000x reference)
"""BoxSampler Trainium2 kernel (8-core SPMD).

Sharding: the B1=50000 proposal axis is split across 8 NeuronCores (6250
each, padded to 6272 = 49 tiles of 128 partitions).  Each core computes its
(6272, 512) slab of the pairwise proposal/GT overlap using the monotone
surrogate

    q = inter / (parea + garea)        (iou = q/(1-q), strictly monotone)

so threshold decisions transfer with adjusted cutoffs (0.7 -> 7/17,
0.3 -> 3/13).  Device outputs per core are only the row-max (per-proposal
best-GT score, for the pos/neg threshold masks) and a per-GT running
column max (used to bound candidate sets).  The exact argmax indices are
resolved on the host in float64 over provably-sufficient candidate sets:

  * target_idx[j] = argmax_i iou[i, j]:  the device winner i* satisfies
    rowmax_dev[i*] >= q_dev[i*, j] = colmax_dev[j], so scanning
    {i : rowmax_dev[i] >= colmax_dev[j]} always contains the argmax —
    regardless of device rounding.  (~1.7M candidate pairs total.)
  * input_idx is only consumed at the 128 selected positive rows, so it is
    recomputed exactly for just those rows (128 x 512).

Device engine split per 128x512 tile (all three vector-capable engines busy):
    DVE:  4x tensor_scalar min/max (2x perf mode), row reduce_max,
          column max-accumulate
    ACT:  2x Relu, table Reciprocal (rs = 1/s)
    GPS:  dx = A-Mx, dy = B-My, u = dxr*dyr, s = garea+parea (broadcast add),
          q = u*rs
"""

import numpy as np

B1 = 50000
B2 = 512
NCORES = 8
PER = B1 // NCORES            # 6250 proposals per core
P = 128                       # partitions
TILES = (PER + P - 1) // P    # 49
PADDED = TILES * P            # 6272
NUM_POS = 128
NUM_NEG = 128

QP = np.float32(0.7 / 1.7)    # iou > 0.7  <=>  q > 7/17
QN = np.float32(0.3 / 1.3)    # iou < 0.3  <=>  q < 3/13

_PROG_CACHE = {}


def _build_program():
    import concourse.bacc as bacc
    import concourse.tile as tile
    import concourse.mybir as mybir

    dt = mybir.dt
    alu = mybir.AluOpType
    act = mybir.ActivationFunctionType

    nc = bacc.Bacc("TRN2", target_bir_lowering=False, debug=False,
                   num_devices=NCORES)

    def act_recip_raw(out, in_):
        """ACT-table reciprocal (~1.2e-5 rel err, measured on HW).

        bass's activation() refuses Reciprocal wholesale; here the error is
        provably tolerable: candidate bounds carry a 1e-4 relative slack and
        threshold decisions within a 3e-5 band of the cutoffs are re-resolved
        exactly on the host.
        """
        eng = nc.scalar
        ins = [eng.lower_ap(in_)] + [
            mybir.ImmediateValue(dtype=dt.float32, value=v)
            for v in (0.0, 1.0, 0.0)  # bias, scale, alpha
        ]
        return eng.add_instruction(
            mybir.InstActivation(
                name=nc.get_next_instruction_name(),
                func=act.Reciprocal,
                ins=ins,
                outs=[eng.lower_ap(out)],
            ))

    boxes = nc.dram_tensor("boxes", [PADDED, 4], dt.float32, kind="ExternalInput")
    gtv = nc.dram_tensor("gtv", [P, 5 * B2], dt.float32, kind="ExternalInput")
    row_q = nc.dram_tensor("row_q", [P, TILES], dt.float32, kind="ExternalOutput")
    # two interleaved column accumulators (even/odd tiles): the odd one is
    # final a tile early, so its output DMA overlaps the last tile's compute
    col_q = nc.dram_tensor("col_q", [P, 2 * B2], dt.float32, kind="ExternalOutput")

    with tile.TileContext(nc) as tc:
        with tc.tile_pool(name="persist", bufs=1) as persist, \
             tc.tile_pool(name="work", bufs=3) as work:

            # ---- one-time setup ----
            boxes_sb = persist.tile([P, TILES, 4], dt.float32)
            nc.sync.dma_start(
                boxes_sb[:], boxes.ap().rearrange("(t p) c -> p t c", p=P))

            # split the GT-vector load so the first tile's ops can start as
            # soon as their slice lands instead of waiting for all 1.25MB
            gtb = persist.tile([P, 5 * B2], dt.float32)
            for v in range(5):
                nc.sync.dma_start(gtb[:, v * B2:(v + 1) * B2],
                                  gtv.ap()[:, v * B2:(v + 1) * B2])
            gx1b = gtb[:, 0 * B2:1 * B2]
            gx0b = gtb[:, 1 * B2:2 * B2]
            gy1b = gtb[:, 2 * B2:3 * B2]
            gy0b = gtb[:, 3 * B2:4 * B2]
            gareab = gtb[:, 4 * B2:5 * B2]

            xc = boxes_sb[:, :, 0]
            yc = boxes_sb[:, :, 1]
            w = boxes_sb[:, :, 2]
            h = boxes_sb[:, :, 3]
            px0a = persist.tile([P, TILES], dt.float32)
            px1a = persist.tile([P, TILES], dt.float32)
            py0a = persist.tile([P, TILES], dt.float32)
            py1a = persist.tile([P, TILES], dt.float32)
            pareaa = persist.tile([P, TILES], dt.float32)
            nc.vector.scalar_tensor_tensor(px1a[:], w, 0.5, xc, alu.mult, alu.add)
            nc.vector.scalar_tensor_tensor(px0a[:], w, -0.5, xc, alu.mult, alu.add)
            nc.vector.scalar_tensor_tensor(py1a[:], h, 0.5, yc, alu.mult, alu.add)
            nc.vector.scalar_tensor_tensor(py0a[:], h, -0.5, yc, alu.mult, alu.add)
            nc.vector.tensor_mul(pareaa[:], w, h)

            colaccA = persist.tile([P, B2], dt.float32)
            colaccB = persist.tile([P, B2], dt.float32)
            nc.vector.memset(colaccA[:], 0.0)
            nc.vector.memset(colaccB[:], 0.0)
            rowq_sb = persist.tile([P, TILES], dt.float32)

            # ---- main loop over proposal tiles ----
            for t in range(TILES):
                px0 = px0a[:, t:t + 1]
                px1 = px1a[:, t:t + 1]
                py0 = py0a[:, t:t + 1]
                py1 = py1a[:, t:t + 1]
                parea = pareaa[:, t:t + 1]

                A = work.tile([P, B2], dt.float32)
                nc.vector.tensor_scalar_min(A[:], gx1b, px1)
                Mx = work.tile([P, B2], dt.float32)
                nc.vector.tensor_scalar_max(Mx[:], gx0b, px0)
                dx = work.tile([P, B2], dt.float32)
                nc.gpsimd.tensor_sub(dx[:], A[:], Mx[:])
                dxr = work.tile([P, B2], dt.float32)
                nc.scalar.activation(dxr[:], dx[:], act.Relu)

                Bt = work.tile([P, B2], dt.float32)
                nc.vector.tensor_scalar_min(Bt[:], gy1b, py1)
                My = work.tile([P, B2], dt.float32)
                nc.vector.tensor_scalar_max(My[:], gy0b, py0)
                dy = work.tile([P, B2], dt.float32)
                nc.gpsimd.tensor_sub(dy[:], Bt[:], My[:])
                dyr = work.tile([P, B2], dt.float32)
                nc.scalar.activation(dyr[:], dy[:], act.Relu)

                u = work.tile([P, B2], dt.float32)
                nc.gpsimd.tensor_mul(u[:], dxr[:], dyr[:])

                s = work.tile([P, B2], dt.float32)
                nc.gpsimd.tensor_add(s[:], gareab, parea.broadcast_to([P, B2]))
                rs = work.tile([P, B2], dt.float32)
                act_recip_raw(rs[:], s[:])

                # q tiles grouped 7-per-buffer (49 = 7x7) so the row-max
                # reduce runs as one 3D [P,7,B2]->[P,7] instruction per group
                # (amortizes the 58-cycle DVE op overhead); pool renaming
                # isolates groups.
                g = t % 7
                if g == 0:
                    qgrp = work.tile([P, 7, B2], dt.float32, tag="qgrp")
                q = qgrp[:, g, :]
                nc.gpsimd.tensor_mul(q, u[:], rs[:])

                if g == 6:
                    nc.vector.tensor_reduce(
                        rowq_sb[:, t - g:t + 1], qgrp[:],
                        mybir.AxisListType.X, alu.max)
                # per-GT running column max (bounds host candidate sets)
                cacc = colaccA if t % 2 == 0 else colaccB
                nc.vector.tensor_tensor(cacc[:], cacc[:], q, alu.max)

            nc.sync.dma_start(row_q.ap(), rowq_sb[:])
            nc.sync.dma_start(col_q.ap()[:, B2:2 * B2], colaccB[:])
            nc.sync.dma_start(col_q.ap()[:, 0:B2], colaccA[:])

    # NOTE: do NOT reorder get_activation_tables() to dodge the second
    # InstLoadActFuncSet — act_func_set_id is an index into act_info.json's
    # original list, so reordering makes HW load the wrong table (sim applies
    # funcs by enum and won't catch it).
    nc.compile()
    return nc


def _get_program():
    if "nc" not in _PROG_CACHE:
        _PROG_CACHE["nc"] = _build_program()
    return _PROG_CACHE["nc"]


def _prep_in_maps(input_boxes, target_boxes):
    ib = np.ascontiguousarray(np.asarray(input_boxes, np.float32)[0])   # (B1,4)
    tb = np.asarray(target_boxes, np.float32)[0]                        # (B2,4)

    half = np.float32(0.5)
    gx0 = tb[:, 0] - tb[:, 2] * half
    gx1 = tb[:, 0] + tb[:, 2] * half
    gy0 = tb[:, 1] - tb[:, 3] * half
    gy1 = tb[:, 1] + tb[:, 3] * half
    garea = tb[:, 2] * tb[:, 3]
    gtv_row = np.concatenate([gx1, gx0, gy1, gy0, garea]).astype(np.float32)
    gtv = np.ascontiguousarray(np.broadcast_to(gtv_row, (P, 5 * B2)))

    pad = np.tile(np.array([-10.0, -10.0, 0.0, 0.0], np.float32),
                  (PADDED - PER, 1))
    in_maps = []
    for c in range(NCORES):
        sl = ib[c * PER:(c + 1) * PER]
        boxes = np.ascontiguousarray(np.concatenate([sl, pad], axis=0))
        in_maps.append({"boxes": boxes, "gtv": gtv})
    return in_maps


def _make_runner():
    """Build a cached multi-core PJRT runner (jitted fn reused across calls)."""
    import jax
    from jax.experimental.shard_map import shard_map
    from jax.sharding import Mesh, PartitionSpec
    import concourse.mybir as mybir
    from concourse.bass2jax import (
        _bass_exec_p, install_neuronx_cc_hook, partition_id_tensor)

    nc = _get_program()
    install_neuronx_cc_hook()

    partition_name = (nc.partition_id_tensor.name
                      if nc.partition_id_tensor else None)
    in_names, out_names, out_avals = [], [], []
    for alloc in nc.m.functions[0].allocations:
        if not isinstance(alloc, mybir.MemoryLocationSet):
            continue
        name = alloc.memorylocations[0].name
        if alloc.kind == "ExternalInput":
            if name != partition_name:
                in_names.append(name)
        elif alloc.kind == "ExternalOutput":
            shape = tuple(alloc.tensor_shape)
            dtype = mybir.dt.np(alloc.dtype)
            out_avals.append(jax.core.ShapedArray(shape, dtype))
            out_names.append(name)
    n_params = len(in_names)
    n_outs = len(out_names)
    all_names = in_names + out_names
    if partition_name is not None:
        all_names.append(partition_name)
    donate = tuple(range(n_params, n_params + n_outs))

    def _body(*args):
        operands = list(args)
        if partition_name is not None:
            operands.append(partition_id_tensor())
        outs = _bass_exec_p.bind(
            *operands,
            out_avals=tuple(out_avals),
            in_names=tuple(all_names),
            out_names=tuple(out_names),
            lowering_input_output_aliases=(),
            sim_require_finite=True,
            sim_require_nnan=True,
            nc=nc,
        )
        return tuple(outs)

    devices = jax.devices()[:NCORES]
    mesh = Mesh(np.asarray(devices), ("core",))
    in_specs = (PartitionSpec("core"),) * (n_params + n_outs)
    out_specs = (PartitionSpec("core"),) * n_outs
    sharded = jax.jit(
        shard_map(_body, mesh=mesh, in_specs=in_specs, out_specs=out_specs,
                  check_rep=False),
        donate_argnums=donate, keep_unused=True)

    def run(in_maps):
        concat_in = [
            np.concatenate([np.asarray(m[name]) for m in in_maps], axis=0)
            for name in in_names
        ]
        concat_zeros = [
            np.zeros((NCORES * a.shape[0], *a.shape[1:]), a.dtype)
            for a in out_avals
        ]
        out_arrs = sharded(*concat_in, *concat_zeros)
        return [
            {name: np.asarray(out_arrs[i]).reshape(NCORES, *out_avals[i].shape)[c]
             for i, name in enumerate(out_names)}
            for c in range(NCORES)
        ]

    return run


def _get_runner():
    if "runner" not in _PROG_CACHE:
        _PROG_CACHE["runner"] = _make_runner()
    return _PROG_CACHE["runner"]


def _run_cores(in_maps, trace=False):
    class _Res:
        pass
    try:
        run = _get_runner()
        r = _Res()
        r.results = run(in_maps)
        r.exec_time_ns = None
        return r
    except Exception:
        from concourse.bass_utils import run_bass_kernel_spmd
        nc = _get_program()
        return run_bass_kernel_spmd(nc, in_maps, core_ids=list(range(NCORES)),
                                    trace=False)


def _geom64(input_boxes, target_boxes):
    ib = np.asarray(input_boxes, np.float32)[0].astype(np.float64)
    tb = np.asarray(target_boxes, np.float32)[0].astype(np.float64)
    px0 = ib[:, 0] - ib[:, 2] / 2
    px1 = ib[:, 0] + ib[:, 2] / 2
    py0 = ib[:, 1] - ib[:, 3] / 2
    py1 = ib[:, 1] + ib[:, 3] / 2
    pa = ib[:, 2] * ib[:, 3]
    gx0 = tb[:, 0] - tb[:, 2] / 2
    gx1 = tb[:, 0] + tb[:, 2] / 2
    gy0 = tb[:, 1] - tb[:, 3] / 2
    gy1 = tb[:, 1] + tb[:, 3] / 2
    ga = tb[:, 2] * tb[:, 3]
    return (px0, px1, py0, py1, pa), (gx0, gx1, gy0, gy1, ga)


def _iou64(pg, gg, rows, j):
    """float64 iou of proposal rows x single GT j."""
    px0, px1, py0, py1, pa = pg
    gx0, gx1, gy0, gy1, ga = gg
    dx = np.minimum(px1[rows], gx1[j]) - np.maximum(px0[rows], gx0[j])
    dy = np.minimum(py1[rows], gy1[j]) - np.maximum(py0[rows], gy0[j])
    inter = np.maximum(dx, 0.0) * np.maximum(dy, 0.0)
    return inter / (pa[rows] + ga[j] - inter)


def _postprocess(results, input_boxes, target_boxes, pos_noise, neg_noise):
    rowq = np.stack([r["row_q"] for r in results])            # (8,128,49)
    # local proposal index p = t*128 + lane  ->  array[lane, t]
    q = np.concatenate([rowq[c].T.reshape(-1)[:PER] for c in range(NCORES)])

    colq = np.stack([r["col_q"] for r in results])            # (8,128,1024)
    colmax_dev = colq.reshape(NCORES, P, 2, B2).max(axis=(0, 1, 2))  # (512,)

    pg, gg = _geom64(input_boxes, target_boxes)

    # --- target_idx: f64 argmax over device-bounded candidate sets ---
    # Slack covers device rounding (recip_approx_fast ~3e-6 rel, plus mults):
    # the true argmax i* has rowmax_dev[i*] >= q_dev[i*,j] >= true_q[i*,j]*(1-eps)
    # >= colmax_dev[j]*(1-eps)/(1+eps), so a 1e-4 relative slack provably
    # keeps it in the candidate set.
    SLACK = 1e-4
    order = np.argsort(q, kind="stable")                      # ascending
    q_sorted = q[order]
    target_idx = np.empty(B2, dtype=np.int64)
    for j in range(B2):
        cut = colmax_dev[j] * (1.0 - SLACK)
        k = np.searchsorted(q_sorted, cut, side="left")
        cand = order[k:]
        if cand.size == 0:
            cand = order[-1:]
        vals = _iou64(pg, gg, cand, j)
        m = vals.max()
        target_idx[j] = cand[vals == m].min()

    # --- masks + sampling ---
    pos = q > QP
    neg = q < QN
    # Device q carries ~1.3e-5 relative error (ACT reciprocal); rows within a
    # 3e-5 band of either cutoff are re-decided exactly in f64 on the host.
    # (On this distribution that is a handful of rows.)
    BAND = 3e-5
    q64d = q.astype(np.float64)
    for thr_q, is_pos in ((float(QP), True), (float(QN), False)):
        amb = np.where(np.abs(q64d - thr_q) < BAND)[0]
        for i in amb:
            vals = _iou64(pg, gg, np.array([i] * B2), np.arange(B2))
            m = vals.max()
            if is_pos:
                pos[i] = m > 0.7
            else:
                neg[i] = m < 0.3
    pos[target_idx] = True
    neg[target_idx] = False
    if neg.sum() == 0:
        neg = ~pos

    pn = np.asarray(pos_noise, np.float32)
    nn = np.asarray(neg_noise, np.float32)
    ps = np.where(pos, pn, np.float32(-1.0))
    ns = np.where(neg, nn, np.float32(-1.0))
    pos_sel = np.argsort(-ps, kind="stable")[:NUM_POS].astype(np.int32)
    neg_sel = np.argsort(-ns, kind="stable")[:NUM_NEG].astype(np.int32)

    # --- input_idx only for the selected positive rows (f64 exact) ---
    px0, px1, py0, py1, pa = pg
    gx0, gx1, gy0, gy1, ga = gg
    r = pos_sel.astype(np.int64)
    dx = np.minimum(px1[r, None], gx1[None, :]) - np.maximum(px0[r, None], gx0[None, :])
    dy = np.minimum(py1[r, None], gy1[None, :]) - np.maximum(py0[r, None], gy0[None, :])
    inter = np.maximum(dx, 0.0) * np.maximum(dy, 0.0)
    iou = inter / (pa[r, None] + ga[None, :] - inter)
    pos_tgt = iou.argmax(axis=1).astype(np.int32)

    return pos_sel, pos_tgt, neg_sel


def kernel(input_boxes, target_boxes, pos_noise, neg_noise):
    in_maps = _prep_in_maps(input_boxes, target_boxes)
    res = _run_cores(in_maps)
    return _postprocess(res.results, input_boxes, target_boxes,
                        pos_noise, neg_noise)
